# revision 25
# baseline (speedup 1.0000x reference)
"""Trainium2 Bass kernel for nn_Conv_M_49409303773352.

Strategy (data-parallel over batch x H-halves -> 8 shards):
  Per position p=(b,h,w): feat = [x-patches(576), m-patches(576)] (3x3, edge pad)
  w1 = feat@W1+b1 [576]; w2 = feat@W2+b2 [4096]
  yr_c = sum_k yp*w1 ; mr_c = sum_k |mp*w1| ; sr_c = sum_k |sp*w1|   (mp,sp>=0)
  y_o  = sum_c yr_c*w2[c,o] ; m_y = (sum_c |mr_c*w2|)/(sum_c |sr_c*w2|)
Device layout: position-tiles of 128 (one image row). GEMM contraction over
channels (64 x + 64 m stacked = 128 partitions) per 3x3 tap k, accumulating in
PSUM; biases folded in via a K=1 ones-row matmul. Apply stage on DVE with
bf16 2x mults + tensor_reduce (abs applied in-reduce).

Host<->device traffic is the wall-clock bottleneck (axon tunnel ~50MB/s), so
inputs are minimized: raw x|m and s rows ship as bf16 and the 3x3 patch
tensors are built ON DEVICE via TensorE transposes (9x smaller upload);
W1/W2 ship as one per-core row-slice [144, 4672] and are assembled with an
on-device AllGather; tap-gathered weight layout is produced by a rearranged
DMA from the gathered DRAM buffer. Outputs return as bf16.
"""
import sys
sys.path.insert(0, '/opt/trn_rl_repo')
import os
import numpy as np
import ml_dtypes

import concourse.bass as bass
import concourse.mybir as mybir
import concourse.tile as tile
from concourse.tile import TileContext
from concourse.vector_clock import ScopedClock
from concourse.bass_utils import run_bass_kernel_spmd
from concourse.masks import make_identity

BF = np.float16
BF_DT = mybir.dt.float16
F32 = mybir.dt.float32

B, C, H, W = 4, 64, 128, 128
ROWS = int(os.environ.get("KERNEL_ROWS", "64"))   # output rows per core
N_CORES = 8
K2 = 9
F1 = 576          # K2*C
F2 = 4096         # C*C
FEAT = 2 * F1     # 1152
WSL = FEAT // N_CORES  # 144 weight rows per core


# ---- walrus only accepts ONE sem wait per instruction: split the final drain
def _split_drain_and_barrier(self, tick_clock, wait_clock):
    nc = self.nc
    probe = nc.sync.nop()
    wait_clock.add_sem_waits(probe.ins, ScopedClock({None: tick_clock.global_clock}))
    waits = list(probe.ins.sync_info.on_wait) if probe.ins.sync_info else []
    if len(waits) > 1:
        probe.ins.sync_info.on_wait = waits[:1]
        for w in waits[1:]:
            extra = nc.sync.nop()
            extra.ins.sync_info = probe.ins.sync_info.__class__(
                on_wait=[w], on_update=[])
    nc.sync.drain()
    nc.all_engine_barrier()
    assert self.sems is not None
    popped = nc._tile_sem_poison_stack.pop()
    assert popped is self._sem_poison
    nc.clear_and_free_semaphores(list(self.sems.allocated().values()))
    nc.all_engine_barrier()


tile.TileContext._drain_and_barrier = _split_drain_and_barrier


def _split_multi_sync(nc):
    """Walrus accepts one sync wait (and update) per instruction: hoist extras
    onto same-engine nops inserted just before (waits) / after (updates)."""
    def make_nop(engine, si_cls, waits=(), updates=()):
        bi = nc.engines[engine].nop()
        blk = nc.cur_bb.bb
        assert blk.instructions[-1] is bi.ins
        blk.instructions.pop()
        bi.ins.sync_info = si_cls(on_wait=list(waits), on_update=list(updates))
        return bi.ins

    for blk in nc.m.functions[0].blocks:
        out = []
        for inst in blk.instructions:
            si = getattr(inst, "sync_info", None)
            if si is None:
                out.append(inst)
                continue
            waits = list(si.on_wait or [])
            updates = list(si.on_update or [])
            extra_w = waits[:-1] if len(waits) > 1 else []
            extra_u = updates[1:] if len(updates) > 1 else []
            if extra_w:
                for w in extra_w:
                    out.append(make_nop(inst.engine, si.__class__, waits=[w]))
                si.on_wait = waits[-1:]
            out.append(inst)
            if extra_u:
                assert inst.opcode not in ("DMACopy", "DMATranspose"), \
                    "cannot defer DMA completion updates"
                si.on_update = updates[:1]
                for u in extra_u:
                    out.append(make_nop(inst.engine, si.__class__, updates=[u]))
        blk.instructions[:] = out


# flat bf16 element offsets of each logical input inside the per-core blob
# (one fused ExternalInput = one h2d transfer; per-array puts cost ~80ms each).
# s ships as fp8e4m3 (only feeds positively-summed |.| reductions, where the
# ~6% element quantization error averages down to <1%): one byte per value,
# so its region occupies _SZ_S3 // 2 bf16 slots.
_SZ_XM = 2 * C * (ROWS + 2) * (W + 2)
_SZ_S3 = C * (ROWS + 2) * (W + 2)
_SZ_WSL = WSL * (F1 + F2)
_OFF_S3 = _SZ_XM
_OFF_WSL = _OFF_S3 + _SZ_S3 // 2
_OFF_B1 = _OFF_WSL + _SZ_WSL
_OFF_B2 = _OFF_B1 + F1
BLOB = _OFF_B2 + F2
F8_DT = mybir.dt.float8e4


def build_program():
    nc = bass.Bass(num_devices=N_CORES)
    blob_d = nc.dram_tensor("blob", [BLOB], BF_DT, kind="ExternalInput")
    xm_d = blob_d[0:_SZ_XM].rearrange(
        "(p r w) -> p r w", p=2 * C, r=ROWS + 2, w=W + 2)
    s3_d = blob_d[_OFF_S3:_OFF_WSL].bitcast(F8_DT).rearrange(
        "(p r w) -> p r w", p=C, r=ROWS + 2, w=W + 2)
    wsl_d = blob_d[_OFF_WSL:_OFF_B1].rearrange(
        "(p f) -> p f", p=WSL, f=F1 + F2)
    b1_d = blob_d[_OFF_B1:_OFF_B2].rearrange("(o f) -> o f", o=1, f=F1)
    b2_d = blob_d[_OFF_B2:BLOB].rearrange("(o f) -> o f", o=1, f=F2)
    # y at [0], m_y at [1] — one output tensor = one d2h fetch + one zero buffer
    out_d = nc.dram_tensor("out", [2, ROWS, W, C], BF_DT, kind="ExternalOutput")

    with TileContext(nc) as tc:
        with (
            tc.tile_pool(name="dramw", bufs=1, space="DRAM") as dramw,
            tc.tile_pool(name="wts", bufs=1) as wts,
            tc.tile_pool(name="rows", bufs=4) as rows,
            tc.tile_pool(name="pats", bufs=3) as pats,
            tc.tile_pool(name="mid", bufs=4) as mid,
            tc.tile_pool(name="sml", bufs=3) as sml,
            tc.tile_pool(name="psw1", bufs=1, space="PSUM") as psw1,
            tc.tile_pool(name="psw2", bufs=2, space="PSUM") as psw2,
            tc.tile_pool(name="pst", bufs=2, space="PSUM") as pst,
        ):
            # ---- weights: per-core slice -> AllGather -> tap-gathered SBUF
            wslc = dramw.tile([WSL, F1 + F2], BF_DT)
            nc.sync.dma_start(out=wslc, in_=wsl_d[:, :])
            wful = dramw.tile([FEAT, F1 + F2], BF_DT)
            nc.gpsimd.collective_compute(
                "AllGather", mybir.AluOpType.bypass,
                replica_groups=[list(range(N_CORES))],
                ins=[wslc[:, :].opt()], outs=[wful[:, :].opt()])
            w1k = wts.tile([128, K2, F1], BF_DT)
            nc.sync.dma_start(
                out=w1k, in_=wful[:, 0:F1].rearrange(
                    "(g c k) f -> (g c) k f", g=2, c=C, k=K2))
            w2k = wts.tile([128, K2, F2], BF_DT)
            nc.sync.dma_start(
                out=w2k, in_=wful[:, F1:F1 + F2].rearrange(
                    "(g c k) f -> (g c) k f", g=2, c=C, k=K2))
            b1s = wts.tile([1, F1], BF_DT)
            nc.sync.dma_start(out=b1s, in_=b1_d[:, :])
            b2s = wts.tile([1, F2], BF_DT)
            nc.sync.dma_start(out=b2s, in_=b2_d[:, :])
            ones = wts.tile([1, 128], BF_DT)
            nc.vector.memset(ones, 1.0)
            ident = wts.tile([128, 128], BF_DT)
            make_identity(nc, ident)

            with tc.For_i(0, ROWS) as h:
                xmr = rows.tile([128, 3, W + 2], BF_DT)
                nc.sync.dma_start(out=xmr, in_=xm_d[:, bass.ds(h, 3), :])
                s3r8 = rows.tile([C, 3, W + 2], F8_DT, tag="s3r8")
                nc.sync.dma_start(out=s3r8, in_=s3_d[:, bass.ds(h, 3), :])
                s3r = rows.tile([C, 3, W + 2], BF_DT, tag="s3r")
                nc.scalar.copy(out=s3r, in_=s3r8)

                # ---- build 3x3 patch tiles on device: [pos, c, k]
                ypt = pats.tile([128, C, K2], BF_DT, tag="ypt")
                mpt = pats.tile([128, C, K2], BF_DT, tag="mpt")
                spt = pats.tile([128, C, K2], BF_DT, tag="spt")
                for k in range(K2):
                    kh, kw = divmod(k, 3)
                    pt = pst.tile([128, 192], BF_DT)
                    nc.tensor.transpose(
                        pt[:, 0:128], xmr[:, kh, kw:kw + 128], ident)
                    nc.tensor.transpose(
                        pt[:, 128:192], s3r[:, kh, kw:kw + 128],
                        ident[0:C, 0:C])
                    nc.scalar.copy(out=ypt[:, :, k], in_=pt[:, 0:C])
                    nc.scalar.copy(out=mpt[:, :, k], in_=pt[:, C:2 * C])
                    nc.vector.tensor_copy(out=spt[:, :, k], in_=pt[:, 2 * C:3 * C])

                # ---- w1 = feat @ W1 + b1  -> PSUM [128 pos, 576]
                ps1 = psw1.tile([128, F1], F32)
                for lo, hi in ((0, 512), (512, F1)):
                    for k in range(K2):
                        kh, kw = divmod(k, 3)
                        nc.tensor.matmul(
                            ps1[:, lo:hi], xmr[:, kh, kw:kw + 128],
                            w1k[:, k, lo:hi], start=(k == 0), stop=False)
                    nc.tensor.matmul(ps1[:, lo:hi], ones[0:1, :],
                                     b1s[0:1, lo:hi], start=False, stop=True)
                w1b = mid.tile([128, F1], BF_DT)
                nc.scalar.copy(out=w1b, in_=ps1)

                # ---- yr/mr/sr: per-position reduce over the 9 taps
                rmap = []
                for ci, (pat, absv) in enumerate(
                        ((ypt, None), (mpt, True), (spt, True))):
                    t1 = mid.tile([128, F1], BF_DT)
                    nc.gpsimd.tensor_mul(
                        t1, pat[:, :, :].rearrange("p c k -> p (c k)"), w1b)
                    red = sml.tile([128, C], F32, tag=f"red{ci}")
                    nc.vector.tensor_reduce(
                        out=red, in_=t1[:, :].rearrange("p (c k) -> p c k", k=K2),
                        axis=mybir.AxisListType.X, op=mybir.AluOpType.add,
                        apply_absolute_value=absv)
                    redb = sml.tile([128, C], BF_DT, tag=f"redb{ci}")
                    nc.scalar.copy(out=redb, in_=red)
                    rmap.append(redb)
                yrb, mrb, srb = rmap

                y_acc = sml.tile([128, C], BF_DT)
                m_acc = sml.tile([128, C], F32)
                s_acc = sml.tile([128, C], F32)

                # ---- w2 = feat @ W2 + b2 in 4 chunks of 1024 cols ([o,c] layout)
                for q in range(4):
                    ps2 = psw2.tile([128, 1024], F32)
                    for j2 in range(2):
                        lo = q * 1024 + j2 * 512
                        for k in range(K2):
                            kh, kw = divmod(k, 3)
                            nc.tensor.matmul(
                                ps2[:, j2 * 512:(j2 + 1) * 512],
                                xmr[:, kh, kw:kw + 128],
                                w2k[:, k, lo:lo + 512], start=(k == 0), stop=False)
                        nc.tensor.matmul(
                            ps2[:, j2 * 512:(j2 + 1) * 512], ones[0:1, :],
                            b2s[0:1, lo:lo + 512], start=False, stop=True)
                    w2b = mid.tile([128, 1024], BF_DT)
                    nc.scalar.copy(out=w2b, in_=ps2)
                    w2v = w2b[:, :].rearrange("p (o c) -> p o c", c=C)
                    for redb, acc, absv in ((yrb, y_acc, None),
                                            (mrb, m_acc, True),
                                            (srb, s_acc, True)):
                        t2 = mid.tile([128, 16, C], BF_DT)
                        bcast = redb[:, :].rearrange(
                            "p (o c) -> p o c", o=1).to_broadcast([128, 16, C])
                        # s-chain always on gpsimd; m-chain alternates to
                        # balance DVE vs gpsimd busy time
                        on_q7 = redb is srb or (redb is mrb and q % 2 == 0)
                        eng = nc.gpsimd if on_q7 else nc.vector
                        eng.tensor_mul(t2, w2v, bcast)
                        with nc.allow_low_precision(
                                reason="bf16 store of f32-internal reduce"):
                            nc.vector.tensor_reduce(
                                out=acc[:, q * 16:(q + 1) * 16], in_=t2,
                                axis=mybir.AxisListType.X, op=mybir.AluOpType.add,
                                apply_absolute_value=absv)

                srec = sml.tile([128, C], F32)
                nc.vector.reciprocal(out=srec, in_=s_acc)
                my_t = sml.tile([128, C], BF_DT)
                nc.vector.tensor_mul(my_t, m_acc, srec)
                nc.sync.dma_start(
                    out=out_d[0, bass.ds(h, 1), :, :].rearrange("r w c -> (r w) c"),
                    in_=y_acc)
                nc.sync.dma_start(
                    out=out_d[1, bass.ds(h, 1), :, :].rearrange("r w c -> (r w) c"),
                    in_=my_t)
    _split_multi_sync(nc)
    return nc


def kernel(x, m, s, W1, b1, W2, b2):
    x = np.asarray(x, np.float32); m = np.asarray(m, np.float32)
    s = np.asarray(s, np.float32)
    W1 = np.asarray(W1, np.float32); W2 = np.asarray(W2, np.float32)
    b1 = np.asarray(b1, np.float32); b2 = np.asarray(b2, np.float32)

    # W2 cols permuted from [c,o] to [o,c]; biases likewise
    W2p = W2.reshape(FEAT, C, C).transpose(0, 2, 1).reshape(FEAT, F2)
    b2p = b2.reshape(C, C).T.reshape(1, F2).astype(BF)
    b1h = b1.reshape(1, F1).astype(BF)
    wcat = np.concatenate([W1, W2p], axis=1).astype(BF)   # [1152, 4672]

    xmp = np.pad(np.concatenate([x, m], axis=1),
                 ((0, 0), (0, 0), (1, 1), (1, 1)), mode='edge').astype(BF)
    smp = np.pad(s, ((0, 0), (0, 0), (1, 1), (1, 1)),
                 mode='edge').astype(mybir.dt.np(F8_DT))

    # one global [N_CORES * BLOB] fp16 array; shard c = core c's fused input
    gblob = np.empty((N_CORES, BLOB), BF)
    shards = []
    for core in range(N_CORES):
        b, half = divmod(core, 2)
        h0 = half * (H // 2)
        shards.append((b, h0))
        gblob[core, :_OFF_S3] = xmp[b, :, h0:h0 + ROWS + 2, :].reshape(-1)
        gblob[core, _OFF_S3:_OFF_WSL].view(np.uint8)[:] = \
            smp[b, :, h0:h0 + ROWS + 2, :].reshape(-1).view(np.uint8)
        gblob[core, _OFF_WSL:_OFF_B1] = wcat[core * WSL:(core + 1) * WSL].reshape(-1)
        gblob[core, _OFF_B1:_OFF_B2] = b1h.reshape(-1)
        gblob[core, _OFF_B2:] = b2p.reshape(-1)
    gblob = gblob.reshape(-1)

    nc = build_program()
    run = _make_runner(nc)
    outs = run(gblob)
    if os.environ.get("KERNEL_TIME"):
        # no NTFF profiling in this axon build: approximate device time by
        # wall-timing repeat executions with the already-compiled
        # executable (includes host I/O, so upper bound); min of 2 to
        # drop tunnel-weather tail noise
        import time
        best = None
        for _ in range(2):
            t0 = time.time()
            run(gblob)
            dt = time.time() - t0
            best = dt if best is None else min(best, dt)
        with open("/tmp/kernel_exec_time.txt", "w") as f:
            f.write(str(int(best * 1e9)))

    y = np.zeros((B, C, H, W), np.float32)
    m_y = np.zeros((B, C, H, W), np.float32)
    for core, (b, h0) in enumerate(shards):
        out = outs[core]          # [2, ROWS, W, C]
        y[b, :, h0:h0 + ROWS, :] = out[0].transpose(2, 0, 1).astype(np.float32)
        m_y[b, :, h0:h0 + ROWS, :] = out[1].transpose(2, 0, 1).astype(np.float32)
    return y, m_y, np.ones_like(m_y)


def _make_runner(nc):
    """Compile nc once and return run(in_maps) -> list of per-core outputs.

    Specialized clone of bass2jax.run_bass_via_pjrt: the jit (and NEFF) is
    built a single time so repeat executions only pay host I/O + dispatch;
    the donated output buffers are zero-filled on device by a side jit
    instead of uploading host zeros.
    """
    import jax
    import jax.numpy as jnp
    from jax.sharding import Mesh, PartitionSpec, NamedSharding
    from jax.experimental.shard_map import shard_map
    from concourse import bass2jax, mybir as _mybir

    bass2jax.install_neuronx_cc_hook()
    partition_name = nc.partition_id_tensor.name if nc.partition_id_tensor else None
    in_names, out_names, out_avals = [], [], []
    for alloc in nc.m.functions[0].allocations:
        if not isinstance(alloc, _mybir.MemoryLocationSet):
            continue
        name = alloc.memorylocations[0].name
        if alloc.kind == "ExternalInput":
            if name != partition_name:
                in_names.append(name)
        elif alloc.kind == "ExternalOutput":
            out_names.append(name)
            out_avals.append(jax.core.ShapedArray(
                tuple(alloc.tensor_shape), _mybir.dt.np(alloc.dtype)))
    n_params = len(in_names)
    n_outs = len(out_avals)
    all_in_names = list(in_names) + out_names
    if partition_name is not None:
        all_in_names.append(partition_name)
    donate = tuple(range(n_params, n_params + n_outs))

    def _body(*args):
        operands = list(args)
        if partition_name is not None:
            operands.append(bass2jax.partition_id_tensor())
        return tuple(bass2jax._bass_exec_p.bind(
            *operands, out_avals=tuple(out_avals), in_names=tuple(all_in_names),
            out_names=tuple(out_names), lowering_input_output_aliases=(),
            sim_require_finite=True, sim_require_nnan=True, nc=nc))

    devices = jax.devices()[:N_CORES]
    mesh = Mesh(np.asarray(devices), ("core",))
    nspec = NamedSharding(mesh, PartitionSpec("core"))
    # No donation: the kernel writes every element of the outputs, so the
    # initial content of the output operands is irrelevant and one
    # device-resident zero buffer can be reused by every execution.
    sharded = jax.jit(
        shard_map(_body, mesh=mesh,
                  in_specs=(PartitionSpec("core"),) * (n_params + n_outs),
                  out_specs=(PartitionSpec("core"),) * n_outs,
                  check_rep=False),
        keep_unused=True)
    zeros = tuple(
        jax.device_put(np.zeros((N_CORES * a.shape[0], *a.shape[1:]), a.dtype),
                       nspec)
        for a in out_avals)

    def run(gblob):
        assert n_params == 1
        dev_in = [jax.device_put(gblob, nspec)]
        out_arrs = sharded(*dev_in, *zeros)
        for o in out_arrs:
            o.copy_to_host_async()
        assert n_outs == 1
        full = np.asarray(out_arrs[0]).reshape(N_CORES, *out_avals[0].shape)
        return [full[c] for c in range(N_CORES)]

    return run
